# revision 7
# baseline (speedup 1.0000x reference)
"""MoBoAligner forward on 8 Trainium2 NeuronCores (Bass/Tile).

Strategy
--------
The reference materializes (B, I, J, K) tensors, but e4[b,i,j,k] depends on k
only through the triangular mask j > k.  Everything collapses to O(B*I*J):

  E[i,j]   = soft-masked noisy energy                       (64, 512)
  LR[i,j]  = log sum_{j'>=j} exp(E[i,j'])   (suffix LSE, per-position max)
  LS[i,k]  = LR[i,k+1]
  alpha DP:  P[i,j] = LSE_{k<j}( alpha[i,k] - LS[i,k] )     (prefix LSE scan)
             alpha[i+1,c] = E[i,c-1] + P[i,c-1]
  delta[i,j] = log( exp(LR[i,j] + P[i,j]) + (J-j) * exp(-10) )
  expanded   = exp(delta)^T @ text

Both LSE scans are computed with running-max normalization via the hardware
tensor_tensor_scan (state = a[t]*state + b[t]), matching the reference's
per-position logsumexp numerics, including the -1000 soft-fill regime.
-inf is represented by the finite sentinel -1e30 (keeps exp/max arithmetic
NaN-free; monotone running maxes guarantee all exp() args are <= 0).

Sharding: pure data parallel over batch B=4; cores 0-3 own batches 0-3,
cores 4-7 duplicate them (outputs ignored).  The Gumbel noise inside the
reference is an input-independent constant: jax.random.uniform(key(42), ...)
computed with the canonical (CPU) threefry implementation.

Self-contained: hardcodes B=4, I=64, J=512, D=256.
"""

import numpy as np

B, I, J, D = 4, 64, 512, 256
NCORES = 8
SENT = -1.0e30  # finite stand-in for -inf

_cache = {}


def _gumbel_u():
    """Canonical jax threefry uniform(key(42), (B,I,J), f32, 1e-20, 1.0) on CPU."""
    if "u" not in _cache:
        import jax

        fn = lambda: jax.random.uniform(
            jax.random.key(42), (B, I, J), jax.numpy.float32, 1e-20, 1.0
        )
        _cache["u"] = np.asarray(jax.jit(fn, backend="cpu")())
    return _cache["u"]


DEBUG = False


def _build_program():
    """Build the per-core Bass program (same program for every core)."""
    if "nc" in _cache:
        return _cache["nc"]

    from contextlib import ExitStack

    import concourse.bacc as bacc
    import concourse.mybir as mybir
    import concourse.tile as tile

    f32 = mybir.dt.float32
    Alu = mybir.AluOpType
    Act = mybir.ActivationFunctionType

    nc = bacc.Bacc(
        "TRN2",
        target_bir_lowering=False,
        debug=False,
        enable_asserts=False,
        num_devices=NCORES,
    )

    textT_d = nc.dram_tensor("textT", [D, I], f32, kind="ExternalInput").ap()
    textN_d = nc.dram_tensor("textN", [I, D], f32, kind="ExternalInput").ap()
    melT_d = nc.dram_tensor("melT", [D, J], f32, kind="ExternalInput").ap()
    mA_d = nc.dram_tensor("mA", [I, J], f32, kind="ExternalInput").ap()
    mC_d = nc.dram_tensor("mC", [I, J], f32, kind="ExternalInput").ap()
    cfill_d = nc.dram_tensor("cfill", [I, J], f32, kind="ExternalInput").ap()
    onehot_d = nc.dram_tensor("onehot", [I, I], f32, kind="ExternalInput").ap()
    ecols_d = nc.dram_tensor("ecols", [1, I * I], f32, kind="ExternalInput").ap()
    shiftT_d = nc.dram_tensor("shiftT", [I, I], f32, kind="ExternalInput").ap()

    delta_d = nc.dram_tensor("delta", [I, J], f32, kind="ExternalOutput").ap()
    expanded_d = nc.dram_tensor("expanded", [J, D], f32, kind="ExternalOutput").ap()
    if DEBUG:
        dbgE_d = nc.dram_tensor("dbgE", [I, J], f32, kind="ExternalOutput").ap()
        dbgLR_d = nc.dram_tensor("dbgLR", [I, J], f32, kind="ExternalOutput").ap()
        dbgF_d = nc.dram_tensor("dbgF", [I, J], f32, kind="ExternalOutput").ap()
        dbgP_d = nc.dram_tensor("dbgP", [I, J], f32, kind="ExternalOutput").ap()
        dbgG_d = nc.dram_tensor("dbgG", [I, J], f32, kind="ExternalOutput").ap()

    with tile.TileContext(nc) as tc, ExitStack() as ctx:
        const = ctx.enter_context(tc.tile_pool(name="const", bufs=1))
        work = ctx.enter_context(tc.tile_pool(name="work", bufs=1))
        dp = ctx.enter_context(tc.tile_pool(name="dp", bufs=3))
        psum = ctx.enter_context(tc.tile_pool(name="psum", bufs=1, space="PSUM"))
        psumr = ctx.enter_context(tc.tile_pool(name="psumr", bufs=2, space="PSUM"))

        # ---- loads ----
        textT0 = const.tile([128, I], f32)
        textT1 = const.tile([128, I], f32)
        nc.sync.dma_start(textT0[:], textT_d[0:128, :])
        nc.sync.dma_start(textT1[:], textT_d[128:256, :])
        melT0 = const.tile([128, J], f32)
        melT1 = const.tile([128, J], f32)
        nc.sync.dma_start(melT0[:], melT_d[0:128, :])
        nc.sync.dma_start(melT1[:], melT_d[128:256, :])
        textN = const.tile([I, D], f32)
        nc.sync.dma_start(textN[:], textN_d[:])
        mA = const.tile([I, J], f32)
        nc.sync.dma_start(mA[:], mA_d[:])
        mC = const.tile([I, J], f32)
        nc.sync.dma_start(mC[:], mC_d[:])
        cfill = const.tile([I, J], f32)
        nc.sync.dma_start(cfill[:], cfill_d[:])
        onehot = const.tile([I, I], f32)
        nc.sync.dma_start(onehot[:], onehot_d[:])
        ecols = const.tile([1, I * I], f32)
        nc.sync.dma_start(ecols[:], ecols_d[:])
        shiftT = const.tile([I, I], f32)
        nc.sync.dma_start(shiftT[:], shiftT_d[:])

        # ---- energy: E = (textT.T @ melT) * mA + mC ----
        E0 = psum.tile([I, J], f32)
        nc.tensor.matmul(E0[:], textT0[:], melT0[:], start=True, stop=False)
        nc.tensor.matmul(E0[:], textT1[:], melT1[:], start=False, stop=True)
        Etmp = work.tile([I, J], f32)
        nc.vector.tensor_tensor(Etmp[:], E0[:], mA[:], Alu.mult)
        E = work.tile([I, J], f32)
        nc.vector.tensor_tensor(E[:], Etmp[:], mC[:], Alu.add)

        # ---- suffix LSE: LR[i,j] = log sum_{j'>=j} exp(E[i,j']) ----
        Msuf = work.tile([I, J], f32)
        nc.vector.tensor_tensor_scan(
            Msuf[:, ::-1], E[:, ::-1], E[:, ::-1], -1e38, Alu.max, Alu.max
        )
        aa = work.tile([I, J], f32)
        nc.vector.memset(aa[:, J - 1 : J], 0.0)
        nc.vector.tensor_tensor(
            aa[:, 0 : J - 1], Msuf[:, 1:J], Msuf[:, 0 : J - 1], Alu.subtract
        )
        nc.scalar.activation(aa[:, 0 : J - 1], aa[:, 0 : J - 1], Act.Exp)
        bb = work.tile([I, J], f32)
        nc.vector.tensor_tensor(bb[:], E[:], Msuf[:], Alu.subtract)
        nc.scalar.activation(bb[:], bb[:], Act.Exp)
        Tt = work.tile([I, J], f32)
        nc.vector.tensor_tensor_scan(
            Tt[:, ::-1], aa[:, ::-1], bb[:, ::-1], 0.0, Alu.mult, Alu.add
        )
        LR = work.tile([I, J], f32)
        nc.scalar.activation(LR[:], Tt[:], Act.Ln)
        nc.vector.tensor_tensor(LR[:], LR[:], Msuf[:], Alu.add)

        # ---- Fmat[i+1, c] = E[i, c-1] - LS[i+1, c]   (LS[i,k] = LR[i,k+1]) ----
        Esh = psum.tile([I, J], f32)
        nc.tensor.matmul(Esh[:], shiftT[:], E[:], start=True, stop=True)
        Fmat = work.tile([I, J], f32)
        nc.vector.memset(Fmat[:, 0:1], SENT)
        nc.vector.memset(Fmat[:, J - 1 : J], SENT)
        nc.vector.tensor_tensor(
            Fmat[:, 1 : J - 1], Esh[:, 0 : J - 2], LR[:, 2:J], Alu.subtract
        )

        # ---- DP over i ----
        Pmat = psum.tile([I, J], f32)
        if DEBUG:
            Gmat = psum.tile([I, J], f32)
        G = dp.tile([1, J], f32, tag="G")
        nc.vector.memset(G[:], SENT)
        nc.vector.tensor_scalar(G[0:1, 0:1], LR[0:1, 1:2], -1.0, None, Alu.mult)

        for i in range(I):
            if DEBUG:
                nc.tensor.matmul(
                    Gmat[:],
                    ecols[0:1, I * i : I * (i + 1)],
                    G[:],
                    start=(i == 0),
                    stop=(i == I - 1),
                    skip_group_check=True,
                )
            Mpre = dp.tile([1, J], f32, tag="Mpre")
            nc.vector.tensor_tensor_scan(Mpre[:], G[:], G[:], -1e38, Alu.max, Alu.max)
            X = dp.tile([1, 2 * J], f32, tag="X")
            nc.vector.tensor_tensor(X[:, 0:J], G[:], Mpre[:], Alu.subtract)
            nc.vector.memset(X[:, J : J + 1], 0.0)
            nc.vector.tensor_tensor(
                X[:, J + 1 : 2 * J], Mpre[:, 0 : J - 1], Mpre[:, 1:J], Alu.subtract
            )
            Y = dp.tile([1, 2 * J], f32, tag="Y")
            nc.scalar.activation(Y[:], X[:], Act.Exp)
            SS = dp.tile([1, J], f32, tag="SS")
            nc.vector.tensor_tensor_scan(
                SS[:], Y[:, J : 2 * J], Y[:, 0:J], 0.0, Alu.mult, Alu.add
            )
            LnS = dp.tile([1, J], f32, tag="LnS")
            nc.scalar.activation(LnS[:], SS[:], Act.Ln)
            Prow = dp.tile([1, J], f32, tag="Prow")
            nc.vector.memset(Prow[:, 0:1], SENT)
            nc.vector.tensor_tensor(
                Prow[:, 1:J], LnS[:, 0 : J - 1], Mpre[:, 0 : J - 1], Alu.add
            )
            nc.tensor.matmul(
                Pmat[:],
                ecols[0:1, I * i : I * (i + 1)],
                Prow[:],
                start=(i == 0),
                stop=(i == I - 1),
                skip_group_check=True,
            )
            if i + 1 < I:
                Frow = psumr.tile([1, J], f32, tag="Frow")
                nc.tensor.matmul(
                    Frow[:],
                    onehot[:, i + 1 : i + 2],
                    Fmat[:],
                    start=True,
                    stop=True,
                    skip_group_check=True,
                )
                G = dp.tile([1, J], f32, tag="G")
                nc.vector.memset(G[:, 0:1], SENT)
                nc.vector.tensor_tensor(
                    G[:, 1:J], Frow[:, 1:J], Prow[:, 0 : J - 1], Alu.add
                )

        if DEBUG:
            nc.sync.dma_start(dbgE_d[:], E[:])
            nc.sync.dma_start(dbgLR_d[:], LR[:])
            nc.sync.dma_start(dbgF_d[:], Fmat[:])
            dbgPs = work.tile([I, J], f32)
            nc.vector.tensor_copy(dbgPs[:], Pmat[:])
            nc.sync.dma_start(dbgP_d[:], dbgPs[:])
            dbgGs = work.tile([I, J], f32)
            nc.vector.tensor_copy(dbgGs[:], Gmat[:])
            nc.sync.dma_start(dbgG_d[:], dbgGs[:])

        # ---- delta = ln(exp(LR + P) + cfill);  w = exp(delta) ----
        A1 = work.tile([I, J], f32)
        nc.vector.tensor_tensor(A1[:], LR[:], Pmat[:], Alu.add)
        nc.scalar.activation(A1[:], A1[:], Act.Exp)
        wmat = work.tile([I, J], f32)
        nc.vector.tensor_tensor(wmat[:], A1[:], cfill[:], Alu.add)
        deltaT = work.tile([I, J], f32)
        nc.scalar.activation(deltaT[:], wmat[:], Act.Ln)
        nc.sync.dma_start(delta_d[:], deltaT[:])

        # ---- expanded = w.T @ text ----
        for q in range(4):
            exq = psumr.tile([128, D], f32, tag="exq")
            nc.tensor.matmul(
                exq[:],
                wmat[:, 128 * q : 128 * (q + 1)],
                textN[:],
                start=True,
                stop=True,
                skip_group_check=True,
            )
            exs = work.tile([128, D], f32, tag="exs")
            nc.scalar.copy(exs[:], exq[:])
            nc.sync.dma_start(expanded_d[128 * q : 128 * (q + 1), :], exs[:])

    nc.compile()
    _cache["nc"] = nc
    return nc


def _prep_inputs(text_embeddings, mel_embeddings, text_mask, mel_mask, temperature_ratio):
    te = np.ascontiguousarray(np.asarray(text_embeddings, np.float32))
    me = np.ascontiguousarray(np.asarray(mel_embeddings, np.float32))
    tm = np.asarray(text_mask).astype(bool)
    mm = np.asarray(mel_mask).astype(bool)
    temp = 0.1 + 0.9 * float(np.asarray(temperature_ratio))

    u = _gumbel_u()
    gnoise = -np.log(-np.log(u.astype(np.float32)))

    ii = np.arange(I)
    jj = np.arange(J)
    Ilen = tm.sum(1)
    lo = ii[:, None] + 1
    hi = J - Ilen[:, None, None] + ii[None, :, None] + 1
    range_ok = (jj[None, None, :] >= lo[None]) & (jj[None, None, :] <= hi)
    ok = range_ok & mm[:, None, :] & tm[:, :, None]

    scale = np.float32(1.0 / (np.sqrt(float(D) * float(D)) * temp))
    mA = np.where(ok, scale, np.float32(0.0)).astype(np.float32)
    mC = np.where(ok, (gnoise / temp).astype(np.float32), np.float32(-1000.0)).astype(
        np.float32
    )
    cfill = np.broadcast_to(
        ((J - jj).astype(np.float32) * np.exp(np.float32(-10.0)))[None, :], (I, J)
    ).astype(np.float32)
    eye = np.eye(I, dtype=np.float32)
    ecols = np.ascontiguousarray(eye.reshape(1, I * I))
    shiftT = np.eye(I, k=1, dtype=np.float32)

    in_maps = []
    for c in range(NCORES):
        b = c % B
        in_maps.append(
            {
                "textT": np.ascontiguousarray(te[b].T),
                "textN": te[b],
                "melT": np.ascontiguousarray(me[b].T),
                "mA": mA[b],
                "mC": mC[b],
                "cfill": np.ascontiguousarray(cfill),
                "onehot": eye,
                "ecols": ecols,
                "shiftT": shiftT,
            }
        )
    return in_maps


def run(inputs, trace=False, trace_cores=None):
    from concourse.bass_utils import run_bass_kernel_spmd

    nc = _build_program()
    in_maps = _prep_inputs(**inputs)
    res = run_bass_kernel_spmd(
        nc,
        in_maps,
        list(range(NCORES)),
        trace=trace,
        trace_cores=trace_cores,
    )
    delta = np.stack([res.results[b]["delta"] for b in range(B)])
    expanded = np.stack([res.results[b]["expanded"] for b in range(B)])
    return (delta, expanded), res


def kernel(**inputs):
    out, _ = run(inputs, trace=False)
    return out
